# revision 1
# baseline (speedup 1.0000x reference)
"""Trainium2 Bass kernel for nn_MultiHeadAttention_50861002719805.

Full inputs in, full output out. Sharding: 8 cores = 4 batches x 2 head-groups
(tensor-parallel over heads, data-parallel over batch). Each core computes
attention for its batch + 8 heads. The pair {2b, 2b+1} exchanges normalized
per-head outputs (bf16 O^T, 512KB per pair, chunked AllGathers overlapped with
compute), then each core projects ALL 16 heads into its own half of the output
columns (host slices W_out columns per core), so no AllReduce is needed.

Per-core algorithm (all in transposed "head-dim on partitions" layout):
  Q^T = (Wq/32)^T x^T        [64,T] per head   (C**-0.5 folded into Wq)
  K^T = Wk^T x^T             [64,T]
  V   = x Wv                 [T,64]
  S^T[s,t] = K^T[:,s].Q^T[:,t]  computed per [128s x 512t] tile, fp32r,
  two heads row-packed on PE row groups 0/64.
  E = exp(S) (no max-shift needed: |S|<~1.5), masked entries := 1.0
    (faithful to the reference bug: masked scores = 1e-9, exp(1e-9)==1.0f)
  Fully-masked s-tiles (s0 >= t0+512) are skipped; their contribution is the
  rank-1 suffix sum_{s>=t0+512} v_aug[s] (incl. Z count), added as a K=1 MM.
  O^T_aug[65,512] = sum_s v_aug[s,:].E[s,t], v_aug = [1 | v] so row 0 = Z.
  O^T_norm = O^T * (1/Z) broadcast, bf16, DMA'd to DRAM, pair-AllGathered.
  out[t, my 512 cols] = sum_jj O_all[:,jj,t]^T @ W_out[jj rows, my cols].
"""
import numpy as np
import ml_dtypes

import concourse.bacc as bacc
import concourse.mybir as mybir
import concourse.tile as tile
from concourse.bass_utils import run_bass_kernel_spmd

F32 = mybir.dt.float32
F32R = mybir.dt.float32r
BF16 = mybir.dt.bfloat16
U8 = mybir.dt.uint8

B, T, D = 4, 2048, 1024
H, HS = 16, 64          # global heads, head size
HL = 8                  # heads per core
TCH, SCH = 512, 128     # t-chunk (psum free dim), s-chunk (partition tile)
NTC, NSC = T // TCH, T // SCH   # 4, 16
NDC = D // 128          # 8 contraction chunks
NP = 4                  # head pairs per core
ADD = mybir.AluOpType.add
MULT = mybir.AluOpType.mult
BYPASS = mybir.AluOpType.bypass
GROUPS = [[0, 1], [2, 3], [4, 5], [6, 7]]


def build(reps=1, collective=True, normalize=True, exp_half=False):
    nc = bacc.Bacc("TRN2", target_bir_lowering=False, debug=False, num_devices=8)

    xT = nc.declare_dram_parameter("xT", [D, T], F32R, isOutput=False)
    wq = nc.declare_dram_parameter("wq", [D, HL * HS], F32R, isOutput=False)
    wk = nc.declare_dram_parameter("wk", [D, HL * HS], F32R, isOutput=False)
    wv = nc.declare_dram_parameter("wv", [D, HL * HS], F32R, isOutput=False)
    wo = nc.declare_dram_parameter("wo", [D, TCH], BF16, isOutput=False)
    mask = nc.declare_dram_parameter("mask", [4, SCH, TCH], U8, isOutput=False)
    out = nc.declare_dram_parameter("out", [T, TCH], F32, isOutput=True)

    with tile.TileContext(nc) as tc:
      for rep in range(reps):
        with (
            tc.tile_pool(name=f"const{rep}", bufs=1) as cpool,
            tc.tile_pool(name=f"wpool{rep}", bufs=1) as wpool,
            tc.tile_pool(name=f"vstp{rep}", bufs=1) as vstp,
            tc.tile_pool(name=f"small{rep}", bufs=2) as sp,
            tc.tile_pool(name=f"dram{rep}", bufs=1, space="DRAM") as dp,
        ):
            o_my = [dp.tile([128, T], BF16, name=f"omy{rep}_{j}") for j in range(NP)]
            o_all = [dp.tile([2, 128, T], BF16, name=f"oall{rep}_{j}") for j in range(NP)]
            # ---- constants ----
            ones_col_bf = cpool.tile([128, 1], BF16)        # chunk-sum lhsT
            ones_t_bf = cpool.tile([128, TCH], BF16)        # masked-fill data
            ones_f = cpool.tile([1, TCH], F32)
            ones_r = cpool.tile([1, TCH], F32R)             # rank-1 rhs
            nc.vector.memset(ones_col_bf[:], 1.0)
            nc.vector.memset(ones_t_bf[:], 1.0)
            nc.vector.memset(ones_f[:], 1.0)
            nc.vector.tensor_copy(ones_r[:], ones_f[:])

            mask_sb = cpool.tile([SCH, 4, TCH], U8)
            for k in range(4):
                nc.sync.dma_start(mask_sb[:, k, :], mask[k, :, :])

            # ---- weights ----
            wq_sb = wpool.tile([128, NDC, HL * HS], F32R)
            wk_sb = wpool.tile([128, NDC, HL * HS], F32R)
            wv_sb = wpool.tile([128, NDC, HL * HS], F32R)
            wo_sb = wpool.tile([128, NDC, TCH], BF16)
            for dc in range(NDC):
                nc.sync.dma_start(wq_sb[:, dc, :], wq[dc * 128:(dc + 1) * 128, :])
                nc.sync.dma_start(wk_sb[:, dc, :], wk[dc * 128:(dc + 1) * 128, :])
                nc.sync.dma_start(wv_sb[:, dc, :], wv[dc * 128:(dc + 1) * 128, :])
                nc.sync.dma_start(wo_sb[:, dc, :], wo[dc * 128:(dc + 1) * 128, :])

            # ---- V phase: V_st[p, sc, h, 0]=1 (Z col), cols 1:65 = v ----
            V_st = vstp.tile([SCH, NSC, HL, HS + 1], BF16)
            nc.vector.memset(V_st[:], 1.0)

            with (
                tc.tile_pool(name=f"xp{rep}", bufs=3) as xp,
                tc.tile_pool(name=f"qkt{rep}", bufs=2) as qkt,
                tc.tile_pool(name=f"ep{rep}", bufs=3) as ep,
            ):
                with tc.tile_pool(name=f"vps{rep}", bufs=1, space="PSUM") as vps:
                    for sub in range(2):
                        pv = [vps.tile([SCH, HL, HS], F32, tag=f"v{i}",
                                       name=f"pv{rep}_{sub}_{i}") for i in range(8)]
                        for dc in range(NDC):
                            xt = xp.tile([128, T], F32R, tag="xqk",
                                         name=f"xtv{rep}_{sub}_{dc}")
                            nc.sync.dma_start(
                                xt[:, 0:T // 2], xT[dc * 128:(dc + 1) * 128,
                                                    sub * 1024:(sub + 1) * 1024])
                            for i in range(8):
                                nc.tensor.matmul(
                                    pv[i][:], xt[:, i * 128:(i + 1) * 128],
                                    wv_sb[:, dc, :],
                                    start=(dc == 0), stop=(dc == NDC - 1))
                        for i in range(8):
                            sc = sub * 8 + i
                            nc.vector.tensor_copy(V_st[:, sc, :, 1:HS + 1], pv[i][:])

                # ---- suffix sums incl. masked-count (col 0 of each head) ----
                HWID = 4 * (HS + 1)  # 260
                vsuf_r = cpool.tile([1, 3, 2, HWID], F32R)
                with tc.tile_pool(name=f"sfps{rep}", bufs=1, space="PSUM") as sfps:
                    for tcb in range(3):
                        for half in range(2):
                            psf = sfps.tile([1, HWID], F32, tag=f"sf{tcb}{half}")
                            lo = 4 * (tcb + 1)
                            for c in range(lo, NSC):
                                nc.tensor.matmul(
                                    psf[:], ones_col_bf[:],
                                    V_st[:, c, half * 4:(half + 1) * 4, :],
                                    start=(c == lo), stop=(c == NSC - 1))
                            nc.vector.tensor_copy(vsuf_r[0:1, tcb, half, :], psf[:])

                for j in range(NP):
                    # -- QK phase for pair j --
                    QT = qkt.tile([128, NTC, TCH], F32R, tag="qt")
                    KT = qkt.tile([128, NTC, TCH], F32R, tag="kt")
                    with tc.tile_pool(name=f"qkps{rep}_{j}", bufs=1,
                                      space="PSUM") as qkps:
                        pq = [qkps.tile([128, TCH], F32, tag=f"q{i}",
                                        name=f"pq{rep}_{j}_{i}") for i in range(NTC)]
                        pk = [qkps.tile([128, TCH], F32, tag=f"k{i}",
                                        name=f"pk{rep}_{j}_{i}") for i in range(NTC)]
                        for dc in range(NDC):
                            xt = xp.tile([128, T], F32R, tag="xqk")
                            nc.sync.dma_start(xt[:], xT[dc * 128:(dc + 1) * 128, :])
                            for tcb in range(NTC):
                                nc.tensor.matmul(
                                    pq[tcb][:],
                                    wq_sb[:, dc, j * 128:(j + 1) * 128],
                                    xt[:, tcb * TCH:(tcb + 1) * TCH],
                                    start=(dc == 0), stop=(dc == NDC - 1))
                                nc.tensor.matmul(
                                    pk[tcb][:],
                                    wk_sb[:, dc, j * 128:(j + 1) * 128],
                                    xt[:, tcb * TCH:(tcb + 1) * TCH],
                                    start=(dc == 0), stop=(dc == NDC - 1))
                        for tcb in range(NTC):
                            nc.vector.tensor_copy(QT[:, tcb, :], pq[tcb][:])
                            nc.vector.tensor_copy(KT[:, tcb, :], pk[tcb][:])

                    # -- attention for heads (2j, 2j+1) --
                    with (
                        tc.tile_pool(name=f"sps{rep}_{j}", bufs=3,
                                     space="PSUM") as spsum,
                        tc.tile_pool(name=f"ops{rep}_{j}", bufs=2,
                                     space="PSUM") as opsum,
                    ):
                        for tcb in range(NTC):
                            nv = 4 * (tcb + 1)   # valid s-chunks
                            E = [ep.tile([SCH, NSC, TCH], BF16, tag="E",
                                         name=f"E{rep}_{j}_{tcb}_{ee}")
                                 for ee in range(2)]
                            po = [opsum.tile([HS + 1, TCH], F32, tag="po",
                                             name=f"po{rep}_{j}_{tcb}_{ee}")
                                  for ee in range(2)]
                            for cp in range(nv // 2):
                                ps = [None, None]
                                for e in range(2):
                                    ps[e] = spsum.tile(
                                        [SCH, 2, TCH], F32, tag="ps",
                                        name=f"ps{rep}_{j}_{tcb}_{cp}_{e}")
                                    for u in range(2):
                                        c = 2 * cp + u
                                        nc.tensor.matmul(
                                            ps[e][:, u, :],
                                            KT[64 * e:64 * e + 64, c // 4,
                                               (c % 4) * SCH:(c % 4 + 1) * SCH],
                                            QT[64 * e:64 * e + 64, tcb, :],
                                            start=True, stop=True)
                                for e in range(2):
                                    # one exp over both chunks (1024 cols)
                                    nc.scalar.activation(
                                        E[e][:, 2 * cp:2 * cp + 2, :], ps[e][:],
                                        mybir.ActivationFunctionType.Exp)
                                    for u in range(2):
                                        c = 2 * cp + u
                                        if c >= 4 * tcb:
                                            nc.vector.copy_predicated(
                                                E[e][:, c, :],
                                                mask_sb[:, c - 4 * tcb, :],
                                                ones_t_bf[:])
                                for e in range(2):
                                    h = 2 * j + e
                                    for u in range(2):
                                        c = 2 * cp + u
                                        nc.tensor.matmul(
                                            po[e][:],
                                            V_st[:, c, h, :],
                                            E[e][:, c, :],
                                            start=(c == 0),
                                            stop=(c == nv - 1 and tcb == 3),
                                            skip_group_check=True)
                            if tcb < 3:
                                for e in range(2):
                                    h = 2 * j + e
                                    nc.tensor.matmul(
                                        po[e][:],
                                        vsuf_r[0:1, tcb, j // 2,
                                               (h % 4) * (HS + 1):
                                               (h % 4 + 1) * (HS + 1)],
                                        ones_r[:],
                                        start=False, stop=True,
                                        skip_group_check=True)
                            # -- normalize: O^T / Z (Z is row 0), off-PSUM --
                            for e in range(2):
                                og = sp.tile([HS + 1, TCH], BF16, tag="og")
                                stg = sp.tile([HS + 1, TCH], F32, tag="stg")
                                nc.vector.tensor_copy(stg[:], po[e][:])
                                if normalize:
                                    rp0 = sp.tile([1, TCH], F32, tag="rp0")
                                    nc.vector.reciprocal(rp0[:], stg[0:1, :])
                                    rbc = sp.tile([HS + 1, TCH], F32, tag="rbc")
                                    nc.gpsimd.partition_broadcast(
                                        rbc[:], rp0[:], channels=HS + 1)
                                    nc.vector.tensor_tensor(
                                        og[:], stg[:], rbc[:], MULT)
                                else:
                                    nc.vector.tensor_copy(og[:], stg[:])
                                nc.sync.dma_start(
                                    o_my[j][64 * e:64 * e + 64,
                                            tcb * TCH:(tcb + 1) * TCH],
                                    og[1:HS + 1, :])

                    # -- exchange this pair's O^T with the partner core --
                    if collective:
                        nc.gpsimd.collective_compute(
                            "AllGather", BYPASS,
                            replica_groups=GROUPS,
                            ins=[o_my[j][:]],
                            outs=[o_all[j][:]],
                        )

            # ---- output projection: all 16 heads x my 512 out columns ----
            with (
                tc.tile_pool(name=f"projp{rep}", bufs=1) as projp,
                tc.tile_pool(name=f"outp{rep}", bufs=3) as outp,
                tc.tile_pool(name=f"pps{rep}", bufs=4, space="PSUM") as pps,
            ):
                O_sb = projp.tile([128, 2, NP, T], BF16)
                for j in range(NP):
                    for g in range(2):
                        src = o_all[j][g, :, :] if collective else o_my[j][:]
                        nc.sync.dma_start(O_sb[:, g, j, :], src)
                for tt in range(T // 128):
                    pp = pps.tile([128, TCH], F32, tag="pp", name=f"pp{rep}_{tt}")
                    for jj in range(NDC):
                        g, j = jj // 4, jj % 4
                        nc.tensor.matmul(
                            pp[:],
                            O_sb[:, g, j, tt * 128:(tt + 1) * 128],
                            wo_sb[:, jj, :],
                            start=(jj == 0), stop=(jj == NDC - 1))
                    ob = outp.tile([128, TCH], F32, tag="ob", name=f"ob{rep}_{tt}")
                    nc.vector.tensor_copy(ob[:], pp[:])
                    nc.sync.dma_start(out[tt * 128:(tt + 1) * 128, :], ob[:])

    nc.compile()
    return nc


def make_mask():
    # mask[k][p, f] = 1 where masked: s > t  <=>  p + 128k > f
    p = np.arange(SCH)[:, None]
    f = np.arange(TCH)[None, :]
    return np.stack([(p + 128 * k > f) for k in range(4)]).astype(np.uint8)


def make_in_maps(x, W_qkv, W_out):
    x = np.asarray(x, dtype=np.float32)
    W_qkv = np.asarray(W_qkv, dtype=np.float32)
    W_out = np.asarray(W_out, dtype=np.float32)
    mask = make_mask()
    in_maps = []
    for c in range(8):
        b, hg = c // 2, c % 2
        heads = slice(hg * HL, (hg + 1) * HL)
        # [h, d, f] -> [d, h, f] -> [d, h*f]
        wq_h = W_qkv[heads, :, 0:HS].transpose(1, 0, 2).reshape(D, HL * HS) * (1.0 / 32.0)
        wk_h = W_qkv[heads, :, HS:2 * HS].transpose(1, 0, 2).reshape(D, HL * HS)
        wv_h = W_qkv[heads, :, 2 * HS:3 * HS].transpose(1, 0, 2).reshape(D, HL * HS)
        in_maps.append({
            "xT": np.ascontiguousarray(x[b].T),
            "wq": np.ascontiguousarray(wq_h),
            "wk": np.ascontiguousarray(wk_h),
            "wv": np.ascontiguousarray(wv_h),
            "wo": np.ascontiguousarray(
                W_out[:, hg * TCH:(hg + 1) * TCH]).astype(ml_dtypes.bfloat16),
            "mask": mask,
        })
    return in_maps


_NC_CACHE = {}


def get_nc():
    if "nc" not in _NC_CACHE:
        _NC_CACHE["nc"] = build()
    return _NC_CACHE["nc"]


def kernel(x, W_qkv, W_out):
    nc = get_nc()
    in_maps = make_in_maps(x, W_qkv, W_out)
    res = run_bass_kernel_spmd(nc, in_maps, list(range(8)))
    out = np.empty((B, T, D), dtype=np.float32)
    for b in range(B):
        out[b, :, 0:TCH] = res.results[2 * b]["out"]
        out[b, :, TCH:D] = res.results[2 * b + 1]["out"]
    return out



# revision 10
# speedup vs baseline: 1.2933x; 1.2933x over previous
"""Trainium2 Bass kernel for nn_MultiHeadAttention_50861002719805.

Full inputs in, full output out. Sharding: 8 cores = 4 batches x 2 head-groups
(tensor-parallel over heads, data-parallel over batch). Each core computes
attention for its batch + 8 heads. The pair {2b, 2b+1} exchanges normalized
per-head outputs (bf16 O^T), then each core projects ALL 16 heads into its own
half of the output columns (host slices W_out columns per core) - no AllReduce.

v2.1: fp8(e4m3) everywhere upstream of the output projection.
  - x^T, Wq*16, Wk*16, Wv*16 quantized to fp8 on host; x^T loaded ONCE into
    SBUF (2MB) and shared by the V and QK phases.
  - QKV projections: fp8 DoubleRow matmuls over d-chunk pairs (0.5 cyc/row).
    QK for pair j+1 is emitted before attention of pair j so the Tensor
    engine fills attention's Act-bound gaps (QK psum pool is only 2 banks).
  - S = K^T Q: fp8 DoubleRow with a zeroed second k-subtile (contraction is
    only 64 deep, but DR still halves the per-row cost).
  - exp folds the reference's C**-0.5 and the 16*16 weight scaling via the
    activation `scale` (1/8192); E stored fp8. exp/S computed only on the
    ~53% of (s,t) blocks at 128x128 granularity that touch s<=t.
  - E.V (po): fp8 DoubleRow over s-chunk pairs; diagonal blocks use exact
    per-128-column ranges.
  - masked (s>t) positions contribute E=1.0 exactly (faithful to the
    reference bug: masked scores 1e-9, exp(1e-9)==1.0f):
      * within a diagonal 128x128 cell: triangular fill (DVE copy_predicated
        and Pool affine_select, alternating)
      * whole masked 128-s-chunks: rank-1 suffix corrections per 128-t-column
        (SUF(c0) = sum of v_aug over s-chunks >= c0, c0=1..16)
  - normalization: 1/Z (DVE) -> Pool partition_broadcast -> DVE multiply.
  - output projection stays bf16 (W_out/16 folded on host).
"""
import numpy as np
import ml_dtypes

import concourse.bacc as bacc
import concourse.mybir as mybir
import concourse.tile as tile
from concourse.bass_utils import run_bass_kernel_spmd

F32 = mybir.dt.float32
BF16 = mybir.dt.bfloat16
FP8 = mybir.dt.float8e4
U8 = mybir.dt.uint8

B, T, D = 4, 2048, 1024
H, HS = 16, 64          # global heads, head size
HL = 8                  # heads per core
TCH, SCH = 512, 128     # t-chunk (psum free dim), s-chunk (partition tile)
NTC, NSC = T // TCH, T // SCH   # 4, 16
NDCP = D // 256         # 4 contraction chunk-pairs (DoubleRow)
NDC = D // 128          # 8 chunks (out proj)
NP = 4                  # head pairs per core
WSCALE = 16.0           # host scales Wq/Wk/Wv by this; exp scale compensates
EXP_SCALE = 1.0 / (WSCALE * WSCALE * 32.0)   # C**-0.5 = 1/32 folded in
MULT = mybir.AluOpType.mult
BYPASS = mybir.AluOpType.bypass
IS_GE = mybir.AluOpType.is_ge
DR = mybir.MatmulPerfMode.DoubleRow
EXP = mybir.ActivationFunctionType.Exp
GROUPS = [[0, 1], [2, 3], [4, 5], [6, 7]]
VW = HS + 2             # v_aug row padded to 66 (16B DoubleRow stride align)
HWID = 4 * VW           # 264: 4 heads' v_aug rows per suffix group


def build(reps=1, collective=True):
    nc = bacc.Bacc("TRN2", target_bir_lowering=False, debug=False, num_devices=8)

    xT = nc.declare_dram_parameter("xT", [D, T], FP8, isOutput=False)
    xTb = nc.declare_dram_parameter("xTb", [D, T], BF16, isOutput=False)
    wq = nc.declare_dram_parameter("wq", [D, HL * HS], BF16, isOutput=False)
    wk = nc.declare_dram_parameter("wk", [D, HL * HS], FP8, isOutput=False)
    wv = nc.declare_dram_parameter("wv", [D, HL * HS], BF16, isOutput=False)
    wo = nc.declare_dram_parameter("wo", [D, TCH], BF16, isOutput=False)
    mask = nc.declare_dram_parameter("mask", [4, SCH, TCH], U8, isOutput=False)
    out = nc.declare_dram_parameter("out", [T, TCH], F32, isOutput=True)

    with tile.TileContext(nc) as tc:
      for rep in range(reps):
        with (
            tc.tile_pool(name=f"const{rep}", bufs=1) as cpool,
            tc.tile_pool(name=f"wpool{rep}", bufs=1) as wpool,
            tc.tile_pool(name=f"vstp{rep}", bufs=1) as vstp,
            tc.tile_pool(name=f"small{rep}", bufs=2) as sp,
            tc.tile_pool(name=f"qkt{rep}", bufs=2) as qkt,
            tc.tile_pool(name=f"ep{rep}", bufs=3) as ep,
            tc.tile_pool(name=f"dram{rep}", bufs=1, space="DRAM") as dp,
            tc.tile_pool(name=f"qkps{rep}", bufs=1, space="PSUM") as qkps,
        ):
            o_my = [dp.tile([128, T], BF16, name=f"omy{rep}_{j}") for j in range(NP)]
            o_all = [dp.tile([2, 128, T], BF16, name=f"oall{rep}_{j}") for j in range(NP)]
            # ---- constants ----
            ones_colb = cpool.tile([128, 1], BF16)         # suffix-sum lhsT
            ones_tb = cpool.tile([128, SCH], BF16)         # cell-fill data
            ones128_bf = cpool.tile([1, SCH], BF16)        # rank-1 prefix rhs
            nc.vector.memset(ones_colb[:], 1.0)
            nc.vector.memset(ones_tb[:], 1.0)
            nc.vector.memset(ones128_bf[:], 1.0)

            tri_sb = cpool.tile([SCH, SCH], U8)            # p > f triangular
            nc.sync.dma_start(tri_sb[:], mask[0, :, 0:SCH])

            # ---- x^T + weights, fp8, DoubleRow-interleaved d-chunk pairs ----
            xp_cm = tc.tile_pool(name=f"xp{rep}", bufs=1)
            xp = xp_cm.__enter__()
            x_sb = xp.tile([128, NDCP, 2, T], FP8)         # K-proj (DR)
            xb_sb = xp.tile([128, NDC, T], BF16)           # Q/V-proj
            wq_sb = wpool.tile([128, NDC, HL * HS], BF16)
            wk_sb = wpool.tile([128, NDCP, 2, HL * HS], FP8)
            wv_sb = wpool.tile([128, NDC, HL * HS], BF16)
            wo_sb = wpool.tile([128, NDC, TCH], BF16)
            for c in range(NDCP):
                for i in range(2):
                    r0 = 256 * c + 128 * i
                    nc.sync.dma_start(x_sb[:, c, i, :], xT[r0:r0 + 128, :])
                    nc.sync.dma_start(wk_sb[:, c, i, :], wk[r0:r0 + 128, :])
            for dc in range(NDC):
                r0 = dc * 128
                nc.sync.dma_start(xb_sb[:, dc, :], xTb[r0:r0 + 128, :])
                nc.sync.dma_start(wq_sb[:, dc, :], wq[r0:r0 + 128, :])
                nc.sync.dma_start(wv_sb[:, dc, :], wv[r0:r0 + 128, :])
                nc.sync.dma_start(wo_sb[:, dc, :], wo[r0:r0 + 128, :])

            qk_tiles = {}

            def emit_qk(j):
                # Q^T,K^T fp8 in [128, 2, T]; k-subtile 1 zeroed (DoubleRow
                # zero-trick). Ring buffers keep their zeros, so only the
                # first two pairs memset.
                QT = qkt.tile([128, 2, T], FP8, tag="qt")
                KT = qkt.tile([128, 2, T], FP8, tag="kt")
                qk_tiles[j] = (QT, KT)
                if j < 2:
                    nc.vector.memset(QT[:, 1, :], 0.0)
                    nc.vector.memset(KT[:, 1, :], 0.0)
                for tcb in range(NTC):
                    pq = qkps.tile([128, TCH], F32, tag="pq",
                                   name=f"pq{rep}_{j}_{tcb}")
                    pk = qkps.tile([128, TCH], F32, tag="pk",
                                   name=f"pk{rep}_{j}_{tcb}")
                    ts = slice(tcb * TCH, (tcb + 1) * TCH)
                    for dc in range(NDC):
                        nc.tensor.matmul(
                            pq[:], wq_sb[:, dc, j * 128:(j + 1) * 128],
                            xb_sb[:, dc, ts],
                            start=(dc == 0), stop=(dc == NDC - 1))
                    for c in range(NDCP):
                        nc.tensor.matmul(
                            pk[:], wk_sb[:, c, :, j * 128:(j + 1) * 128],
                            x_sb[:, c, :, ts], perf_mode=DR,
                            start=(c == 0), stop=(c == NDCP - 1))
                    nc.vector.tensor_copy(QT[:, 0, ts], pq[:])
                    nc.vector.tensor_copy(KT[:, 0, ts], pk[:])

            # QK for pair 0 first: its 2 psum banks coexist with the V
            # phase's 4, and attention 0 can start as soon as V/SUF land.
            emit_qk(0)

            # ---- V phase: V_st[s, sc, h, 0]=1 (Z col), cols 1:65 = 16*v ----
            V_st = vstp.tile([SCH, NSC, HL, VW], BF16)
            nc.vector.memset(V_st[:, :, :, 0:1], 1.0)
            nc.vector.memset(V_st[:, :, :, HS + 1:VW], 0.0)

            with tc.tile_pool(name=f"vps{rep}", bufs=4, space="PSUM") as vps:
                for sc in range(NSC):
                    pv = vps.tile([SCH, HL, HS], F32, tag="pv",
                                  name=f"pv{rep}_{sc}")
                    for dc in range(NDC):
                        nc.tensor.matmul(
                            pv[:], xb_sb[:, dc, sc * 128:(sc + 1) * 128],
                            wv_sb[:, dc, :],
                            start=(dc == 0), stop=(dc == NDC - 1))
                    nc.vector.tensor_copy(V_st[:, sc, :, 1:HS + 1], pv[:])

            # ---- SUF(c0) = sum_{c>=c0} chunkSum(c), c0=1..16 (idx c0-1);
            #      idx 15 stays zero. Snapshots of a descending psum accum. ----
            suf_bf = cpool.tile([1, NSC, 2, HWID], BF16)
            nc.vector.memset(suf_bf[:], 0.0)
            with tc.tile_pool(name=f"sufp{rep}", bufs=1, space="PSUM") as sufps:
                psf = [sufps.tile([1, HWID], F32, tag=f"suf{h}",
                                  name=f"psf{rep}_{h}") for h in range(2)]
                for c in range(NSC - 1, 0, -1):
                    for half in range(2):
                        nc.tensor.matmul(
                            psf[half][:], ones_colb[:],
                            V_st[:, c, half * 4:(half + 1) * 4, :],
                            start=(c == NSC - 1), stop=(c == 1),
                            skip_group_check=True)
                    for half in range(2):
                        nc.vector.tensor_copy(
                            suf_bf[0:1, c - 1, half, :], psf[half][:])

            with (
                tc.tile_pool(name=f"sps{rep}", bufs=2, space="PSUM") as spsum,
                tc.tile_pool(name=f"ops{rep}", bufs=2, space="PSUM") as opsum,
            ):
                for j in range(NP):
                    if j + 1 < NP:
                        emit_qk(j + 1)   # overlaps attention of pair j
                    QT, KT = qk_tiles.pop(j)
                    # -- attention for heads (2j, 2j+1) --
                    for tcb in range(NTC):
                        d0 = 4 * tcb          # first diagonal chunk
                        ts = slice(tcb * TCH, (tcb + 1) * TCH)
                        E = [ep.tile([SCH, NSC, TCH], BF16, tag="E",
                                     name=f"E{rep}_{j}_{tcb}_{ee}")
                             for ee in range(2)]
                        po = [opsum.tile([HS + 1, TCH], F32, tag="po",
                                         name=f"po{rep}_{j}_{tcb}_{ee}")
                              for ee in range(2)]
                        # S + exp, 2 chunks per psum tile
                        for e in range(2):
                            hp = slice(64 * e, 64 * e + 64)
                            for cp in range(2 * tcb):   # non-diagonal pairs
                                ps = spsum.tile(
                                    [SCH, 2, TCH], F32, tag="ps",
                                    name=f"ps{rep}_{j}_{tcb}_{e}_{cp}")
                                for u in range(2):
                                    c = 2 * cp + u
                                    nc.tensor.matmul(
                                        ps[:, u, :],
                                        KT[hp, :, c * 128:(c + 1) * 128],
                                        QT[hp, :, ts],
                                        perf_mode=DR,
                                        start=True, stop=True)
                                nc.scalar.activation(
                                    E[e][:, 2 * cp:2 * cp + 2, :], ps[:],
                                    EXP, scale=EXP_SCALE)
                            # diagonal chunks d0+k: k0,k1 full range;
                            # k2,k3 on t in [256:512) only
                            ps = spsum.tile([SCH, 2, TCH], F32, tag="ps",
                                            name=f"psd{rep}_{j}_{tcb}_{e}0")
                            for k in range(2):
                                c = d0 + k
                                nc.tensor.matmul(
                                    ps[:, k, :],
                                    KT[hp, :, c * 128:(c + 1) * 128],
                                    QT[hp, :, ts],
                                    perf_mode=DR, start=True, stop=True)
                            nc.scalar.activation(
                                E[e][:, d0:d0 + 2, :], ps[:],
                                EXP, scale=EXP_SCALE)
                            ps = spsum.tile([SCH, 2, TCH], F32, tag="ps",
                                            name=f"psd{rep}_{j}_{tcb}_{e}1")
                            for k in range(2):
                                c = d0 + 2 + k
                                nc.tensor.matmul(
                                    ps[:, k, 256:TCH],
                                    KT[hp, :, c * 128:(c + 1) * 128],
                                    QT[hp, :, tcb * TCH + 256:
                                       tcb * TCH + TCH],
                                    perf_mode=DR, start=True, stop=True)
                            nc.scalar.activation(
                                E[e][:, d0 + 2:d0 + 4, 256:TCH],
                                ps[:, 0:2, 256:TCH], EXP, scale=EXP_SCALE)
                            # triangular cell fills: E=1.0 where p>f
                            # within cell (d0+k, t-sub k)
                            for k in range(4):
                                cell = E[e][:, d0 + k,
                                            128 * k:128 * k + 128]
                                if k % 2 == 0:
                                    nc.vector.copy_predicated(
                                        cell, tri_sb[:], ones_tb[:])
                                else:
                                    # keep where f - p >= 0 (s <= t)
                                    nc.gpsimd.affine_select(
                                        cell, cell,
                                        pattern=[[1, SCH]],
                                        compare_op=IS_GE,
                                        fill=1.0, base=0,
                                        channel_multiplier=-1)
                        # po accumulation: E.v_aug (bf16, per chunk;
                        # diagonal chunks on their exact valid t-ranges)
                        for e in range(2):
                            h = 2 * j + e
                            for c in range(4 * tcb):    # non-diag chunks
                                nc.tensor.matmul(
                                    po[e][:],
                                    V_st[:, c, h, 0:HS + 1],
                                    E[e][:, c, :],
                                    start=(c == 0), stop=False,
                                    skip_group_check=True)
                            for k in range(4):
                                t0 = 128 * k
                                nc.tensor.matmul(
                                    po[e][:, t0:TCH],
                                    V_st[:, d0 + k, h, 0:HS + 1],
                                    E[e][:, d0 + k, t0:TCH],
                                    start=(tcb == 0 and k == 0), stop=False,
                                    skip_group_check=True)
                            # rank-1 masked-suffix per 128-t-column:
                            # + SUF(4*tcb + jj + 1) (idx 15 is zeros)
                            for jj in range(4):
                                c0 = 4 * tcb + jj + 1
                                nc.tensor.matmul(
                                    po[e][:, 128 * jj:128 * jj + 128],
                                    suf_bf[0:1, c0 - 1, j // 2,
                                           (h % 4) * VW:
                                           (h % 4) * VW + HS + 1],
                                    ones128_bf[:],
                                    start=False, stop=(jj == 3),
                                    skip_group_check=True)
                        # -- normalize: O^T/Z (Z is row 0) --
                        for e in range(2):
                            rz = sp.tile([1, TCH], F32, tag="rz")
                            nc.vector.reciprocal(rz[:], po[e][0:1, :])
                            rbc = sp.tile([HS + 1, TCH], F32, tag="rbc")
                            nc.gpsimd.partition_broadcast(
                                rbc[:], rz[:], channels=HS + 1)
                            og = sp.tile([HS + 1, TCH], BF16, tag="og")
                            nc.vector.tensor_tensor(
                                og[:], po[e][:], rbc[:], MULT)
                            nc.sync.dma_start(
                                o_my[j][64 * e:64 * e + 64, ts],
                                og[1:HS + 1, :])

                    # -- exchange this pair's O^T with the partner core --
                    if collective:
                        nc.gpsimd.collective_compute(
                            "AllGather", BYPASS,
                            replica_groups=GROUPS,
                            ins=[o_my[j][:]],
                            outs=[o_all[j][:]],
                        )

            xp_cm.__exit__(None, None, None)

            # ---- output projection: all 16 heads x my 512 out columns ----
            with (
                tc.tile_pool(name=f"projp{rep}", bufs=1) as projp,
                tc.tile_pool(name=f"outp{rep}", bufs=3) as outp,
                tc.tile_pool(name=f"pps{rep}", bufs=4, space="PSUM") as pps,
            ):
                O_sb = projp.tile([128, 2, NP, T], BF16)
                for j in range(NP):
                    for g in range(2):
                        src = o_all[j][g, :, :] if collective else o_my[j][:]
                        nc.sync.dma_start(O_sb[:, g, j, :], src)
                for tt in range(T // 128):
                    pp = pps.tile([128, TCH], F32, tag="pp", name=f"pp{rep}_{tt}")
                    for jj in range(NDC):
                        g, jp = jj // 4, jj % 4
                        nc.tensor.matmul(
                            pp[:],
                            O_sb[:, g, jp, tt * 128:(tt + 1) * 128],
                            wo_sb[:, jj, :],
                            start=(jj == 0), stop=(jj == NDC - 1))
                    ob = outp.tile([128, TCH], F32, tag="ob", name=f"ob{rep}_{tt}")
                    nc.vector.tensor_copy(ob[:], pp[:])
                    nc.sync.dma_start(out[tt * 128:(tt + 1) * 128, :], ob[:])

    nc.compile()
    return nc


def make_mask():
    # mask[k][p, f] = 1 where masked: s > t  <=>  p + 128k > f
    p = np.arange(SCH)[:, None]
    f = np.arange(TCH)[None, :]
    return np.stack([(p + 128 * k > f) for k in range(4)]).astype(np.uint8)


def make_in_maps(x, W_qkv, W_out):
    x = np.asarray(x, dtype=np.float32)
    W_qkv = np.asarray(W_qkv, dtype=np.float32)
    W_out = np.asarray(W_out, dtype=np.float32)
    mask = make_mask()
    fp8 = ml_dtypes.float8_e4m3
    in_maps = []
    for c in range(8):
        b, hg = c // 2, c % 2
        heads = slice(hg * HL, (hg + 1) * HL)
        # [h, d, f] -> [d, h, f] -> [d, h*f]; x16 scale for fp8 range
        wq_h = W_qkv[heads, :, 0:HS].transpose(1, 0, 2).reshape(D, HL * HS)
        wk_h = W_qkv[heads, :, HS:2 * HS].transpose(1, 0, 2).reshape(D, HL * HS)
        wv_h = W_qkv[heads, :, 2 * HS:3 * HS].transpose(1, 0, 2).reshape(D, HL * HS)
        bf = ml_dtypes.bfloat16
        in_maps.append({
            "xT": np.ascontiguousarray(x[b].T).astype(fp8),
            "xTb": np.ascontiguousarray(x[b].T).astype(bf),
            "wq": np.ascontiguousarray(wq_h * WSCALE).astype(bf),
            "wk": np.ascontiguousarray(wk_h * WSCALE).astype(fp8),
            "wv": np.ascontiguousarray(wv_h * WSCALE).astype(bf),
            "wo": np.ascontiguousarray(
                W_out[:, hg * TCH:(hg + 1) * TCH] / WSCALE
            ).astype(ml_dtypes.bfloat16),
            "mask": mask,
        })
    return in_maps


_NC_CACHE = {}


def get_nc():
    if "nc" not in _NC_CACHE:
        _NC_CACHE["nc"] = build()
    return _NC_CACHE["nc"]


def kernel(x, W_qkv, W_out):
    nc = get_nc()
    in_maps = make_in_maps(x, W_qkv, W_out)
    res = run_bass_kernel_spmd(nc, in_maps, list(range(8)))
    out = np.empty((B, T, D), dtype=np.float32)
    for b in range(B):
        out[b, :, 0:TCH] = res.results[2 * b]["out"]
        out[b, :, TCH:D] = res.results[2 * b + 1]["out"]
    return out
